# revision 8
# baseline (speedup 1.0000x reference)
"""Trainium2 Bass kernel for the Powderworld BehaviorFluidFlow step.

Contract: kernel(**inputs) takes the FULL unsharded inputs
  world         (16, 20, 512, 512) f32
  rand_movement (16, 1, 512, 512) f32
  rand_interact (16, 1, 512, 512) f32   (unused by the reference)
  rand_element  (16, 1, 512, 512) f32   (unused by the reference)
and returns the FULL (16, 20, 512, 512) f32 output.

Sharding: data-parallel over batch; core k processes batches [2k, 2k+1].
All roll-based neighbor access is along W (axis 3), which stays local.

Architecture: per (batch, 128-row h-tile) the channels live in two haloed
SBUF tiles: TD (128, 16, 514) = [id, dens, grav, mom, didg, EN | 10
payload] blended on DVE, TP (128, 5, 514) = 5 payload channels blended on
Pool.  EN = (is_element & ~did_gravity) | is_air is PRECOMPUTED once from
the loaded ids and blended through pass 1 exactly like data (the flag is a
per-pixel function of channels that move together, so its blend is exact);
this removes the pass-2 membership and two mask-chain merges.  Each pass
computes one a-mask ("pixel j takes its in-direction neighbor"); the
b-mask is the a-mask at a shifted column (the move is a pairwise swap).
DVE channels ping-pong TD->TN->TD with a batched Activation base-copy
(hoisted to overlap the mask chain) + two batched copy_predicated per
chunk; Pool channels move in place with an exact XOR pair-swap
(f = a32 & (x ^ x_nbr); x ^= f; x ^= f_shift) -- no base copy.  Mask-chain
two-input merges run on Pool, membership and the mask finisher (which
writes the int8 CP mask directly) on DVE.
"""
import sys

if '/opt/trn_rl_repo' not in sys.path:
    sys.path.insert(0, '/opt/trn_rl_repo')

import numpy as np
import concourse.bacc as bacc
import concourse.mybir as mybir
import concourse.tile as tile
from concourse.bass_utils import run_bass_kernel_spmd

A = mybir.AluOpType
F32 = mybir.dt.float32
I8 = mybir.dt.int8
I32 = mybir.dt.int32

B, C, H, W = 16, 20, 512, 512
N_CORES = 8
BPC = B // N_CORES
P = 128

_nc_cache = {}

NTD = 16       # TD slots: 5 mask chs + EN + 10 payload
NPOOL = 5      # pool XOR-blended channels (payload only)
ENC = 5        # EN channel index in TD/TN

# TD slot -> dram channel: [0,1,2,6,8, EN, 3,4,5, 7, 9,10,11,12,13,14]
TD_RUNS = [(0, 3, 0), (6, 7, 3), (8, 9, 4), (3, 6, 6), (7, 8, 9), (9, 15, 10)]
TP_RUNS = [(15, 20, 0)]

# membership set {empty, water, lava, gas, acid, agentK, agentL}
# = ids {0, 3, 8, 9, 12, 14, 15} = bits of 54025
MBITS = 54025


def build_kernel(bpc=BPC, c=C, h=H, w=W):
    key = (bpc, c, h, w)
    if key in _nc_cache:
        return _nc_cache[key]

    nc = bacc.Bacc("TRN2", target_bir_lowering=False, debug=False,
                   num_devices=N_CORES)
    world = nc.dram_tensor("world", [bpc, c, h, w], F32, kind="ExternalInput")
    rand = nc.dram_tensor("rand", [bpc, h, w], F32, kind="ExternalInput")
    out = nc.dram_tensor("out", [bpc, c, h, w], F32, kind="ExternalOutput")

    WH = w + 2          # haloed width; data in cols [1, w], halos at 0, w+1
    n_ht = h // P
    MAIN = slice(1, w + 1)
    LEFT = slice(0, w)
    RIGHT = slice(2, w + 2)

    iters = [(b, t) for b in range(bpc) for t in range(n_ht)]
    n = len(iters)
    st = [dict() for _ in range(n)]

    with tile.TileContext(nc) as tc:
        with tc.tile_pool(name="td", bufs=2) as tdp, \
             tc.tile_pool(name="tp", bufs=2) as tpp, \
             tc.tile_pool(name="tn", bufs=2) as tnp, \
             tc.tile_pool(name="rp", bufs=2) as rp, \
             tc.tile_pool(name="am", bufs=3) as amp, \
             tc.tile_pool(name="mk", bufs=5) as mk, \
             tc.tile_pool(name="mh", bufs=3) as mh, \
             tc.tile_pool(name="it", bufs=2) as itp, \
             tc.tile_pool(name="px", bufs=2) as pxp:

            def membership(ch0, out_tile, out_slice=None):
                """out = 1 where id in bits(MBITS) else 0 (exact int trick).

                (id+127)<<23 is the f32 bit pattern of 2^id; converting back
                to int gives 1<<id; AND with MBITS + nonzero test.
                """
                IT = itp.tile([P, w], I32, tag="it", name="IT")
                VT = itp.tile([P, w], I32, tag="it", name="VT")
                nc.vector.tensor_copy(IT[:], ch0)
                nc.vector.tensor_scalar(IT[:], IT[:], 8388608, 1065353216,
                                        A.mult, A.add)
                nc.vector.tensor_copy(VT[:], IT[:].bitcast(F32))
                tgt = out_tile if out_slice is None else out_slice
                nc.vector.tensor_scalar(tgt, VT[:], MBITS, 0,
                                        A.bitwise_and, A.is_gt)

            def loads(i):
                b, t = iters[i]
                hs = slice(t * P, (t + 1) * P)
                s = st[i]
                s['TD'] = tdp.tile([P, NTD, WH], F32, tag="td", name=f"TD{i}")
                s['TP'] = tpp.tile([P, NPOOL, WH], F32, tag="tp", name=f"TP{i}")
                s['RAND'] = rp.tile([P, w], F32, tag="rand", name=f"RAND{i}")
                TD, TP = s['TD'], s['TP']
                for d0, d1, t0 in TD_RUNS:
                    nc.sync.dma_start(
                        TD[:, t0:t0 + (d1 - d0), MAIN],
                        world[b, d0:d1, hs, :].rearrange("c p w -> p c w"))
                for d0, d1, t0 in TP_RUNS:
                    nc.sync.dma_start(
                        TP[:, t0:t0 + (d1 - d0), MAIN],
                        world[b, d0:d1, hs, :].rearrange("c p w -> p c w"))
                nc.sync.dma_start(s['RAND'][:], rand[b, hs, :])

            def prep(i):
                """EN channel + halo columns of the loaded tiles.  The EN
                compute (DVE) and halo copies (Act) depend only on load(i)."""
                s = st[i]
                TD, TP = s['TD'], s['TP']
                cid = TD[:, 0, MAIN]
                E = mk.tile([P, w], F32, tag="mk", name="E")
                membership(cid, E)
                # EN = (E & didg<0.5) | id>13.5
                nc.vector.scalar_tensor_tensor(E[:], TD[:, 4, MAIN], 0.5,
                                               E[:], A.is_lt, A.logical_and)
                nc.vector.scalar_tensor_tensor(TD[:, ENC, MAIN], cid, 13.5,
                                               E[:], A.is_gt, A.logical_or)
                nc.scalar.copy(TD[:, :, 0:1], TD[:, :, w:w + 1])
                nc.scalar.copy(TD[:, :, w + 1:w + 2], TD[:, :, 1:2])
                nc.scalar.copy(TP[:, :, 0:1], TP[:, :, w:w + 1])
                nc.scalar.copy(TP[:, :, w + 1:w + 2], TP[:, :, 1:2])

            def base1(i):
                # Act: TN <- TD; no mask dependence, overlaps the mask chain
                s = st[i]
                s['TN'] = tnp.tile([P, NTD, WH], F32, tag="tn", name=f"TN{i}")
                nc.scalar.copy(s['TN'][:, 0:6, :], s['TD'][:, 0:6, :])
                nc.scalar.copy(s['TN'][:, 6:NTD, :], s['TD'][:, 6:NTD, :])

            def base2(i):
                # Act: TD <- TN, skipping the EN slot
                s = st[i]
                nc.scalar.copy(s['TD'][:, 0:5, :], s['TN'][:, 0:5, :])
                nc.scalar.copy(s['TD'][:, 6:NTD, :], s['TN'][:, 6:NTD, :])

            def mask_pass(i, p2):
                """a-mask for a pass.  p2=False: fall-left (neighbor j-1,
                overlap shift +1); p2=True: fall-right (neighbor j+1,
                overlap shift -1)."""
                s = st[i]
                cur = s['TD'] if not p2 else s['TN']
                nbr = LEFT if not p2 else RIGHT
                RAND = s['RAND']
                FS = mk.tile([P, w], F32, tag="mk", name="FS")
                DN = mk.tile([P, w], F32, tag="mk", name="DN")
                GB = mk.tile([P, w], F32, tag="mk", name="GB")
                LH = mh.tile([P, WH], F32, tag="mkh", name="LH")
                AM8 = amp.tile([P, WH], I8, tag="am8", name=f"AM8{p2}_{i}")

                # --- Pool: the two-input merges --------------------------
                nc.gpsimd.tensor_tensor(FS[:], RAND[:], cur[:, 3, MAIN],
                                        A.add)
                if p2:
                    # + nfm = 2*b1 = 2*a1(j+1)
                    nc.gpsimd.scalar_tensor_tensor(
                        FS[:], s['A18'][:, RIGHT], 2.0, FS[:], A.mult, A.add)
                nc.gpsimd.tensor_tensor(DN[:], cur[:, 1, MAIN], cur[:, 1, nbr],
                                        A.is_gt)
                cmp_op = A.is_gt if not p2 else A.is_le
                nc.gpsimd.scalar_tensor_tensor(DN[:], FS[:], 0.5, DN[:],
                                               cmp_op, A.logical_and)
                nc.gpsimd.tensor_tensor(GB[:], cur[:, 2, MAIN], cur[:, 2, nbr],
                                        A.logical_and)
                nc.gpsimd.tensor_tensor(DN[:], DN[:], cur[:, ENC, MAIN],
                                        A.logical_and)
                nc.gpsimd.tensor_tensor(LH[:, MAIN], DN[:], GB[:],
                                        A.logical_and)
                # --- halo of L, overlap removal -> int8 mask on DVE ------
                which = 2 if p2 else 1
                if not p2:
                    nc.scalar.copy(LH[:, w + 1:w + 2], LH[:, 1:2])
                    nc.vector.scalar_tensor_tensor(AM8[:, MAIN], LH[:, RIGHT],
                                                   0.0, LH[:, MAIN],
                                                   A.is_equal, A.logical_and)
                    nc.scalar.copy(AM8[:, w + 1:w + 2], AM8[:, 1:2])
                    vs = slice(1, w + 2)
                else:
                    nc.scalar.copy(LH[:, 0:1], LH[:, w:w + 1])
                    nc.vector.scalar_tensor_tensor(AM8[:, MAIN], LH[:, LEFT],
                                                   0.0, LH[:, MAIN],
                                                   A.is_equal, A.logical_and)
                    nc.scalar.copy(AM8[:, 0:1], AM8[:, w:w + 1])
                    vs = slice(0, w + 1)
                s[f'A{which}8'] = AM8
                # int32 all-ones mask for the pool XOR channels
                A32 = amp.tile([P, WH], I32, tag="a32", name=f"A32{which}_{i}")
                nc.vector.tensor_scalar(A32[:, vs], AM8[:, vs], -1, None,
                                        A.mult)
                s[f'A32_{which}'] = A32

            def blend_cps(i, p2):
                """Two batched copy_predicated per chunk (DVE), plus halo
                refreshes (Act) split by chunk so pass-2 unblocks early."""
                s = st[i]
                if not p2:
                    src, dst = s['TD'], s['TN']
                    AM8 = s['A18']
                    am, bm = AM8[:, MAIN], AM8[:, RIGHT]   # b[j] = a[j+1]
                    asrc, bsrc = LEFT, RIGHT
                    chunks = ((0, 6, 0), (6, NTD, 6))
                else:
                    src, dst = s['TN'], s['TD']
                    AM8 = s['A28']
                    am, bm = AM8[:, MAIN], AM8[:, LEFT]    # b[j] = a[j-1]
                    asrc, bsrc = RIGHT, LEFT
                    chunks = ((0, 5, 0), (6, NTD, 6))
                for c0, c1, d0 in chunks:
                    nch = c1 - c0
                    d1 = d0 + nch
                    amb = am.unsqueeze(1).broadcast_to((P, nch, w))
                    bmb = bm.unsqueeze(1).broadcast_to((P, nch, w))
                    nc.vector.copy_predicated(dst[:, d0:d1, MAIN], amb,
                                              src[:, c0:c1, asrc])
                    nc.vector.copy_predicated(dst[:, d0:d1, MAIN], bmb,
                                              src[:, c0:c1, bsrc])
                    if not p2:
                        # refresh halos for the pass-2 chain/CPs; mask chunk
                        # first so mask_pass(p2) unblocks early.
                        nc.scalar.copy(dst[:, d0:d1, 0:1],
                                       dst[:, d0:d1, w:w + 1])
                        nc.scalar.copy(dst[:, d0:d1, w + 1:w + 2],
                                       dst[:, d0:d1, 1:2])

            def blend_pool(i, p2):
                """Exact in-place XOR pair-swap of the TP channels.
                pass1 pairs (j-1, j) via a1[j]; pass2 pairs (j, j+1)."""
                s = st[i]
                TP = s['TP']
                which = 2 if p2 else 1
                A32 = s[f'A32_{which}']
                ti = TP[:, :, :].bitcast(I32)
                X = pxp.tile([P, NPOOL, WH], I32, tag="px", name=f"X{which}")
                if not p2:
                    fs, fo = slice(1, w + 2), slice(0, w + 1)
                    osl = RIGHT
                else:
                    fs, fo = slice(0, w + 1), slice(1, w + 2)
                    osl = LEFT
                a32 = A32[:, fs].unsqueeze(1).broadcast_to((P, NPOOL, w + 1))
                nc.gpsimd.tensor_tensor(X[:, :, fs], ti[:, :, fs],
                                        ti[:, :, fo], A.bitwise_xor)
                nc.gpsimd.tensor_tensor(X[:, :, fs], X[:, :, fs], a32,
                                        A.bitwise_and)
                nc.gpsimd.tensor_tensor(ti[:, :, MAIN], ti[:, :, MAIN],
                                        X[:, :, MAIN], A.bitwise_xor)
                nc.gpsimd.tensor_tensor(ti[:, :, MAIN], ti[:, :, MAIN],
                                        X[:, :, osl], A.bitwise_xor)
                if not p2:
                    nc.scalar.copy(TP[:, :, 0:1], TP[:, :, w:w + 1])
                    nc.scalar.copy(TP[:, :, w + 1:w + 2], TP[:, :, 1:2])

            def fixup_store(i):
                b, t = iters[i]
                hs = slice(t * P, (t + 1) * P)
                s = st[i]
                TD, TP = s['TD'], s['TP']
                NF = mk.tile([P, w], F32, tag="mk", name="NF")
                FLI8 = amp.tile([P, w], I8, tag="fli8", name=f"FLI8_{i}")
                # nfm = 2*b1 - 2*b2 = 2*(a1[j+1] - a2[j-1])
                nc.vector.tensor_tensor(NF[:], s['A18'][:, RIGHT],
                                        s['A28'][:, LEFT], A.subtract)
                nc.vector.tensor_scalar(NF[:], NF[:], 2.0, None, A.mult)
                membership(TD[:, 0, MAIN], None, out_slice=FLI8[:])
                nc.vector.copy_predicated(TD[:, 3, MAIN], FLI8[:], NF[:])
                for d0, d1, t0 in TD_RUNS:
                    nc.scalar.dma_start(
                        out[b, d0:d1, hs, :].rearrange("c p w -> p c w"),
                        TD[:, t0:t0 + (d1 - d0), MAIN])
                for d0, d1, t0 in TP_RUNS:
                    nc.scalar.dma_start(
                        out[b, d0:d1, hs, :].rearrange("c p w -> p c w"),
                        TP[:, t0:t0 + (d1 - d0), MAIN])

            # ---- software-pipelined emission -------------------------------
            loads(0)
            prep(0)
            base1(0)
            for i in range(n):
                mask_pass(i, False)
                blend_pool(i, False)
                blend_cps(i, False)
                if i + 1 < n:
                    loads(i + 1)
                mask_pass(i, True)
                base2(i)
                blend_pool(i, True)
                blend_cps(i, True)
                fixup_store(i)
                if i + 1 < n:
                    prep(i + 1)
                    base1(i + 1)

    nc.compile()
    _nc_cache[key] = nc
    return nc


def kernel(world, rand_movement, rand_interact, rand_element):
    del rand_interact, rand_element
    nc = build_kernel()
    in_maps = []
    for k in range(N_CORES):
        bs = slice(k * BPC, (k + 1) * BPC)
        in_maps.append({
            "world": np.ascontiguousarray(world[bs]),
            "rand": np.ascontiguousarray(rand_movement[bs, 0]),
        })
    res = run_bass_kernel_spmd(nc, in_maps, list(range(N_CORES)))
    return np.concatenate([res.results[k]["out"] for k in range(N_CORES)], axis=0)


# revision 13
# speedup vs baseline: 1.2609x; 1.2609x over previous
"""Trainium2 Bass kernel for the Powderworld BehaviorFluidFlow step.

Contract: kernel(**inputs) takes the FULL unsharded inputs
  world         (16, 20, 512, 512) f32
  rand_movement (16, 1, 512, 512) f32
  rand_interact (16, 1, 512, 512) f32   (unused by the reference)
  rand_element  (16, 1, 512, 512) f32   (unused by the reference)
and returns the FULL (16, 20, 512, 512) f32 output.

Sharding: data-parallel over batch; core k processes batches [2k, 2k+1].
All roll-based neighbor access is along W (axis 3), which stays local.

Architecture: per (batch, 128-row h-tile) the channels live in two haloed
SBUF tiles: TD (128, 16, 514) = [id, dens, grav, mom, didg, EN | 10
payload] blended on DVE, TP (128, 5, 514) = 5 payload channels blended on
Pool.  EN = (is_element & ~did_gravity) | is_air is PRECOMPUTED once from
the loaded ids and blended through pass 1 exactly like data (the flag is a
per-pixel function of channels that move together, so its blend is exact);
this removes the pass-2 membership and two mask-chain merges.  Each pass
computes one a-mask ("pixel j takes its in-direction neighbor"); the
b-mask is the a-mask at a shifted column (the move is a pairwise swap).
DVE channels ping-pong TD->TN->TD with a batched Activation base-copy
(hoisted to overlap the mask chain) + two batched copy_predicated per
chunk; Pool channels move in place with an exact XOR pair-swap
(f = a32 & (x ^ x_nbr); x ^= f; x ^= f_shift) -- no base copy.  Mask-chain
two-input merges run on Pool, membership and the mask finisher (which
writes the int8 CP mask directly) on DVE.
"""
import sys

if '/opt/trn_rl_repo' not in sys.path:
    sys.path.insert(0, '/opt/trn_rl_repo')

import numpy as np
import concourse.bacc as bacc
import concourse.mybir as mybir
import concourse.tile as tile
from concourse.bass_utils import run_bass_kernel_spmd

A = mybir.AluOpType
F32 = mybir.dt.float32
I8 = mybir.dt.int8
I32 = mybir.dt.int32

B, C, H, W = 16, 20, 512, 512
N_CORES = 8
BPC = B // N_CORES
P = 128

_nc_cache = {}

NTD = 16       # TD slots: 5 mask chs + EN + 10 payload
NPOOL = 5      # pool XOR-blended channels (payload only)
ENC = 5        # EN channel index in TD/TN

# TD slot -> dram channel: [0,1,2,6,8, EN, 3,4,5, 7, 9,10,11,12,13,14]
TD_RUNS = [(0, 3, 0), (6, 7, 3), (8, 9, 4), (3, 6, 6), (7, 8, 9), (9, 15, 10)]
TP_RUNS = [(15, 20, 0)]

# membership set {empty, water, lava, gas, acid, agentK, agentL}
# = ids {0, 3, 8, 9, 12, 14, 15} = bits of 54025
MBITS = 54025


def build_kernel(bpc=BPC, c=C, h=H, w=W):
    key = (bpc, c, h, w)
    if key in _nc_cache:
        return _nc_cache[key]

    nc = bacc.Bacc("TRN2", target_bir_lowering=False, debug=False,
                   num_devices=N_CORES)
    world = nc.dram_tensor("world", [bpc, c, h, w], F32, kind="ExternalInput")
    rand = nc.dram_tensor("rand", [bpc, h, w], F32, kind="ExternalInput")
    out = nc.dram_tensor("out", [bpc, c, h, w], F32, kind="ExternalOutput")

    WH = w + 2          # haloed width; data in cols [1, w], halos at 0, w+1
    n_ht = h // P
    MAIN = slice(1, w + 1)
    LEFT = slice(0, w)
    RIGHT = slice(2, w + 2)

    iters = [(b, t) for b in range(bpc) for t in range(n_ht)]
    n = len(iters)
    st = [dict() for _ in range(n)]

    with tile.TileContext(nc) as tc:
        with tc.tile_pool(name="td", bufs=2) as tdp, \
             tc.tile_pool(name="tp", bufs=2) as tpp, \
             tc.tile_pool(name="tn", bufs=2) as tnp, \
             tc.tile_pool(name="rp", bufs=2) as rp, \
             tc.tile_pool(name="am", bufs=3) as amp, \
             tc.tile_pool(name="mk", bufs=5) as mk, \
             tc.tile_pool(name="mh", bufs=3) as mh, \
             tc.tile_pool(name="it", bufs=2) as itp, \
             tc.tile_pool(name="px", bufs=2) as pxp:

            def membership(ch0, out_tile, out_slice=None):
                """out = 1 where id in bits(MBITS) else 0 (exact int trick).

                (id+127)<<23 is the f32 bit pattern of 2^id; converting back
                to int gives 1<<id; AND with MBITS + nonzero test.
                """
                IT = itp.tile([P, w], I32, tag="it", name="IT")
                VT = itp.tile([P, w], I32, tag="it", name="VT")
                nc.vector.tensor_copy(IT[:], ch0)
                nc.vector.tensor_scalar(IT[:], IT[:], 8388608, 1065353216,
                                        A.mult, A.add)
                nc.vector.tensor_copy(VT[:], IT[:].bitcast(F32))
                tgt = out_tile if out_slice is None else out_slice
                nc.vector.tensor_scalar(tgt, VT[:], MBITS, 0,
                                        A.bitwise_and, A.is_gt)

            def loads(i):
                b, t = iters[i]
                hs = slice(t * P, (t + 1) * P)
                s = st[i]
                s['TD'] = tdp.tile([P, NTD, WH], F32, tag="td", name=f"TD{i}")
                s['TP'] = tpp.tile([P, NPOOL, WH], F32, tag="tp", name=f"TP{i}")
                s['RAND'] = rp.tile([P, w], F32, tag="rand", name=f"RAND{i}")
                TD, TP = s['TD'], s['TP']
                for d0, d1, t0 in TD_RUNS:
                    nc.sync.dma_start(
                        TD[:, t0:t0 + (d1 - d0), MAIN],
                        world[b, d0:d1, hs, :].rearrange("c p w -> p c w"))
                for d0, d1, t0 in TP_RUNS:
                    nc.sync.dma_start(
                        TP[:, t0:t0 + (d1 - d0), MAIN],
                        world[b, d0:d1, hs, :].rearrange("c p w -> p c w"))
                nc.sync.dma_start(s['RAND'][:], rand[b, hs, :])

            def prep(i):
                """EN channel + halo columns of the loaded tiles.  The EN
                compute (DVE) and halo copies (Act) depend only on load(i)."""
                s = st[i]
                TD, TP = s['TD'], s['TP']
                cid = TD[:, 0, MAIN]
                E = mk.tile([P, w], F32, tag="mk", name="E")
                membership(cid, E)
                # EN = (E & didg<0.5) | id>13.5
                nc.vector.scalar_tensor_tensor(E[:], TD[:, 4, MAIN], 0.5,
                                               E[:], A.is_lt, A.logical_and)
                nc.vector.scalar_tensor_tensor(TD[:, ENC, MAIN], cid, 13.5,
                                               E[:], A.is_gt, A.logical_or)
                nc.scalar.copy(TD[:, :, 0:1], TD[:, :, w:w + 1])
                nc.scalar.copy(TD[:, :, w + 1:w + 2], TD[:, :, 1:2])
                nc.scalar.copy(TP[:, :, 0:1], TP[:, :, w:w + 1])
                nc.scalar.copy(TP[:, :, w + 1:w + 2], TP[:, :, 1:2])

            def base1(i):
                # Act: TN <- TD; no mask dependence, overlaps the mask chain
                s = st[i]
                s['TN'] = tnp.tile([P, NTD, WH], F32, tag="tn", name=f"TN{i}")
                nc.scalar.copy(s['TN'][:, 0:6, :], s['TD'][:, 0:6, :])
                nc.scalar.copy(s['TN'][:, 6:NTD, :], s['TD'][:, 6:NTD, :])

            def base2_chunk(i, which):
                # Act: TD <- TN (skipping the EN slot); emitted per chunk
                # right after that chunk's pass-1 CPs and halos so the big
                # copies interleave with CP progress instead of blocking.
                s = st[i]
                if which == 0:
                    nc.scalar.copy(s['TD'][:, 0:5, :], s['TN'][:, 0:5, :])
                else:
                    nc.scalar.copy(s['TD'][:, 6:NTD, :], s['TN'][:, 6:NTD, :])

            def mask_pass(i, p2):
                """a-mask for a pass.  p2=False: fall-left (neighbor j-1,
                overlap shift +1); p2=True: fall-right (neighbor j+1,
                overlap shift -1)."""
                s = st[i]
                cur = s['TD'] if not p2 else s['TN']
                nbr = LEFT if not p2 else RIGHT
                RAND = s['RAND']
                FS = mk.tile([P, w], F32, tag="mk", name="FS")
                DN = mk.tile([P, w], F32, tag="mk", name="DN")
                GB = mk.tile([P, w], F32, tag="mk", name="GB")
                LH = mh.tile([P, WH], F32, tag="mkh", name="LH")
                AM8 = amp.tile([P, WH], I8, tag="am8", name=f"AM8{p2}_{i}")

                # --- Pool: the two-input merges --------------------------
                nc.gpsimd.tensor_tensor(FS[:], RAND[:], cur[:, 3, MAIN],
                                        A.add)
                if p2:
                    # + nfm = 2*b1 = 2*a1(j+1)
                    nc.gpsimd.scalar_tensor_tensor(
                        FS[:], s['A18'][:, RIGHT], 2.0, FS[:], A.mult, A.add)
                nc.gpsimd.tensor_tensor(DN[:], cur[:, 1, MAIN], cur[:, 1, nbr],
                                        A.is_gt)
                cmp_op = A.is_gt if not p2 else A.is_le
                nc.gpsimd.scalar_tensor_tensor(DN[:], FS[:], 0.5, DN[:],
                                               cmp_op, A.logical_and)
                nc.gpsimd.tensor_tensor(GB[:], cur[:, 2, MAIN], cur[:, 2, nbr],
                                        A.logical_and)
                nc.gpsimd.tensor_tensor(DN[:], DN[:], cur[:, ENC, MAIN],
                                        A.logical_and)
                nc.gpsimd.tensor_tensor(LH[:, MAIN], DN[:], GB[:],
                                        A.logical_and)
                # --- halo of L, overlap removal -> int8 mask, all on DVE
                # (tiny copies stay off the Act queue, which holds the big
                # base copies and must not head-block the mask finisher)
                which = 2 if p2 else 1
                if not p2:
                    nc.vector.tensor_copy(LH[:, w + 1:w + 2], LH[:, 1:2])
                    nc.vector.scalar_tensor_tensor(AM8[:, MAIN], LH[:, RIGHT],
                                                   0.0, LH[:, MAIN],
                                                   A.is_equal, A.logical_and)
                    nc.vector.tensor_copy(AM8[:, w + 1:w + 2], AM8[:, 1:2])
                    vs = slice(1, w + 2)
                else:
                    nc.vector.tensor_copy(LH[:, 0:1], LH[:, w:w + 1])
                    nc.vector.scalar_tensor_tensor(AM8[:, MAIN], LH[:, LEFT],
                                                   0.0, LH[:, MAIN],
                                                   A.is_equal, A.logical_and)
                    nc.vector.tensor_copy(AM8[:, 0:1], AM8[:, w:w + 1])
                    vs = slice(0, w + 1)
                s[f'A{which}8'] = AM8
                # int32 all-ones mask for the pool XOR channels
                A32 = amp.tile([P, WH], I32, tag="a32", name=f"A32{which}_{i}")
                nc.vector.tensor_scalar(A32[:, vs], AM8[:, vs], -1, None,
                                        A.mult)
                s[f'A32_{which}'] = A32

            def blend_cps(i, p2):
                """Two batched copy_predicated per chunk (DVE), plus halo
                refreshes (Act) split by chunk so pass-2 unblocks early."""
                s = st[i]
                if not p2:
                    src, dst = s['TD'], s['TN']
                    AM8 = s['A18']
                    am, bm = AM8[:, MAIN], AM8[:, RIGHT]   # b[j] = a[j+1]
                    asrc, bsrc = LEFT, RIGHT
                    chunks = ((0, 6, 0), (6, NTD, 6))
                else:
                    src, dst = s['TN'], s['TD']
                    AM8 = s['A28']
                    am, bm = AM8[:, MAIN], AM8[:, LEFT]    # b[j] = a[j-1]
                    asrc, bsrc = RIGHT, LEFT
                    chunks = ((0, 5, 0), (6, NTD, 6))
                for ci, (c0, c1, d0) in enumerate(chunks):
                    nch = c1 - c0
                    d1 = d0 + nch
                    amb = am.unsqueeze(1).broadcast_to((P, nch, w))
                    bmb = bm.unsqueeze(1).broadcast_to((P, nch, w))
                    nc.vector.copy_predicated(dst[:, d0:d1, MAIN], amb,
                                              src[:, c0:c1, asrc])
                    nc.vector.copy_predicated(dst[:, d0:d1, MAIN], bmb,
                                              src[:, c0:c1, bsrc])
                    if not p2:
                        # refresh halos for the pass-2 chain/CPs; mask chunk
                        # first so mask_pass(p2) unblocks early.
                        nc.scalar.copy(dst[:, d0:d1, 0:1],
                                       dst[:, d0:d1, w:w + 1])
                        nc.scalar.copy(dst[:, d0:d1, w + 1:w + 2],
                                       dst[:, d0:d1, 1:2])
                        base2_chunk(i, ci)

            def blend_pool(i, p2):
                """Exact in-place XOR pair-swap of the TP channels.
                pass1 pairs (j-1, j) via a1[j]; pass2 pairs (j, j+1)."""
                s = st[i]
                TP = s['TP']
                which = 2 if p2 else 1
                A32 = s[f'A32_{which}']
                ti = TP[:, :, :].bitcast(I32)
                X = pxp.tile([P, NPOOL, WH], I32, tag="px", name=f"X{which}")
                if not p2:
                    fs, fo = slice(1, w + 2), slice(0, w + 1)
                    osl = RIGHT
                else:
                    fs, fo = slice(0, w + 1), slice(1, w + 2)
                    osl = LEFT
                a32 = A32[:, fs].unsqueeze(1).broadcast_to((P, NPOOL, w + 1))
                nc.gpsimd.tensor_tensor(X[:, :, fs], ti[:, :, fs],
                                        ti[:, :, fo], A.bitwise_xor)
                nc.gpsimd.tensor_tensor(X[:, :, fs], X[:, :, fs], a32,
                                        A.bitwise_and)
                nc.gpsimd.tensor_tensor(ti[:, :, MAIN], ti[:, :, MAIN],
                                        X[:, :, MAIN], A.bitwise_xor)
                nc.gpsimd.tensor_tensor(ti[:, :, MAIN], ti[:, :, MAIN],
                                        X[:, :, osl], A.bitwise_xor)

            def tp_halos(i):
                TP = st[i]['TP']
                nc.scalar.copy(TP[:, :, 0:1], TP[:, :, w:w + 1])
                nc.scalar.copy(TP[:, :, w + 1:w + 2], TP[:, :, 1:2])

            def fixup_store(i):
                b, t = iters[i]
                hs = slice(t * P, (t + 1) * P)
                s = st[i]
                TD, TP = s['TD'], s['TP']
                NF = mk.tile([P, w], F32, tag="mk", name="NF")
                FLI8 = amp.tile([P, w], I8, tag="fli8", name=f"FLI8_{i}")
                # nfm = 2*b1 - 2*b2 = 2*(a1[j+1] - a2[j-1])
                nc.vector.tensor_tensor(NF[:], s['A18'][:, RIGHT],
                                        s['A28'][:, LEFT], A.subtract)
                nc.vector.tensor_scalar(NF[:], NF[:], 2.0, None, A.mult)
                membership(TD[:, 0, MAIN], None, out_slice=FLI8[:])
                nc.vector.copy_predicated(TD[:, 3, MAIN], FLI8[:], NF[:])
                for d0, d1, t0 in TD_RUNS:
                    nc.scalar.dma_start(
                        out[b, d0:d1, hs, :].rearrange("c p w -> p c w"),
                        TD[:, t0:t0 + (d1 - d0), MAIN])
                for d0, d1, t0 in TP_RUNS:
                    nc.scalar.dma_start(
                        out[b, d0:d1, hs, :].rearrange("c p w -> p c w"),
                        TP[:, t0:t0 + (d1 - d0), MAIN])

            # ---- software-pipelined emission -------------------------------
            loads(0)
            prep(0)
            base1(0)
            for i in range(n):
                mask_pass(i, False)
                blend_pool(i, False)
                blend_cps(i, False)     # also emits base2 chunks on Act
                tp_halos(i)
                if i + 1 < n:
                    loads(i + 1)
                    prep(i + 1)
                    base1(i + 1)
                mask_pass(i, True)
                blend_pool(i, True)
                blend_cps(i, True)
                fixup_store(i)

    nc.compile()
    _nc_cache[key] = nc
    return nc


def kernel(world, rand_movement, rand_interact, rand_element):
    del rand_interact, rand_element
    nc = build_kernel()
    in_maps = []
    for k in range(N_CORES):
        bs = slice(k * BPC, (k + 1) * BPC)
        in_maps.append({
            "world": np.ascontiguousarray(world[bs]),
            "rand": np.ascontiguousarray(rand_movement[bs, 0]),
        })
    res = run_bass_kernel_spmd(nc, in_maps, list(range(N_CORES)))
    return np.concatenate([res.results[k]["out"] for k in range(N_CORES)], axis=0)


# revision 14
# speedup vs baseline: 1.2672x; 1.0050x over previous
"""Trainium2 Bass kernel for the Powderworld BehaviorFluidFlow step.

Contract: kernel(**inputs) takes the FULL unsharded inputs
  world         (16, 20, 512, 512) f32
  rand_movement (16, 1, 512, 512) f32
  rand_interact (16, 1, 512, 512) f32   (unused by the reference)
  rand_element  (16, 1, 512, 512) f32   (unused by the reference)
and returns the FULL (16, 20, 512, 512) f32 output.

Sharding: data-parallel over batch; core k processes batches [2k, 2k+1].
All roll-based neighbor access is along W (axis 3), which stays local.

Architecture: per (batch, 128-row h-tile) the channels live in two haloed
SBUF tiles: TD (128, 16, 514) = [id, dens, grav, mom, didg, EN | 10
payload] blended on DVE, TP (128, 5, 514) = 5 payload channels blended on
Pool.  EN = (is_element & ~did_gravity) | is_air is PRECOMPUTED once from
the loaded ids and blended through pass 1 exactly like data (the flag is a
per-pixel function of channels that move together, so its blend is exact);
this removes the pass-2 membership and two mask-chain merges.  Each pass
computes one a-mask ("pixel j takes its in-direction neighbor"); the
b-mask is the a-mask at a shifted column (the move is a pairwise swap).
DVE channels ping-pong TD->TN->TD with a batched Activation base-copy
(hoisted to overlap the mask chain) + two batched copy_predicated per
chunk; Pool channels move in place with an exact XOR pair-swap
(f = a32 & (x ^ x_nbr); x ^= f; x ^= f_shift) -- no base copy.  Mask-chain
two-input merges run on Pool, membership and the mask finisher (which
writes the int8 CP mask directly) on DVE.
"""
import sys

if '/opt/trn_rl_repo' not in sys.path:
    sys.path.insert(0, '/opt/trn_rl_repo')

import numpy as np
import concourse.bacc as bacc
import concourse.mybir as mybir
import concourse.tile as tile
from concourse.bass_utils import run_bass_kernel_spmd

A = mybir.AluOpType
F32 = mybir.dt.float32
I8 = mybir.dt.int8
I32 = mybir.dt.int32

B, C, H, W = 16, 20, 512, 512
N_CORES = 8
BPC = B // N_CORES
P = 128

_nc_cache = {}

NTD = 16       # TD slots: 5 mask chs + EN + 10 payload
NPOOL = 5      # pool XOR-blended channels (payload only)
ENC = 5        # EN channel index in TD/TN

# TD slot -> dram channel: [0,1,2,6,8, EN, 3,4,5, 7, 9,10,11,12,13,14]
TD_RUNS = [(0, 3, 0), (6, 7, 3), (8, 9, 4), (3, 6, 6), (7, 8, 9), (9, 15, 10)]
TP_RUNS = [(15, 20, 0)]

# membership set {empty, water, lava, gas, acid, agentK, agentL}
# = ids {0, 3, 8, 9, 12, 14, 15} = bits of 54025
MBITS = 54025


def build_kernel(bpc=BPC, c=C, h=H, w=W):
    key = (bpc, c, h, w)
    if key in _nc_cache:
        return _nc_cache[key]

    nc = bacc.Bacc("TRN2", target_bir_lowering=False, debug=False,
                   num_devices=N_CORES)
    world = nc.dram_tensor("world", [bpc, c, h, w], F32, kind="ExternalInput")
    rand = nc.dram_tensor("rand", [bpc, h, w], F32, kind="ExternalInput")
    out = nc.dram_tensor("out", [bpc, c, h, w], F32, kind="ExternalOutput")

    WH = w + 2          # haloed width; data in cols [1, w], halos at 0, w+1
    n_ht = h // P
    MAIN = slice(1, w + 1)
    LEFT = slice(0, w)
    RIGHT = slice(2, w + 2)

    iters = [(b, t) for b in range(bpc) for t in range(n_ht)]
    n = len(iters)
    st = [dict() for _ in range(n)]

    with tile.TileContext(nc) as tc:
        with tc.tile_pool(name="td", bufs=2) as tdp, \
             tc.tile_pool(name="tp", bufs=2) as tpp, \
             tc.tile_pool(name="tn", bufs=2) as tnp, \
             tc.tile_pool(name="rp", bufs=2) as rp, \
             tc.tile_pool(name="am", bufs=3) as amp, \
             tc.tile_pool(name="mk", bufs=5) as mk, \
             tc.tile_pool(name="mh", bufs=3) as mh, \
             tc.tile_pool(name="it", bufs=2) as itp, \
             tc.tile_pool(name="px", bufs=2) as pxp:

            def membership(ch0, out_tile, out_slice=None):
                """out = 1 where id in bits(MBITS) else 0 (exact int trick).

                (id+127)<<23 is the f32 bit pattern of 2^id; converting back
                to int gives 1<<id; AND with MBITS + nonzero test.
                """
                IT = itp.tile([P, w], I32, tag="it", name="IT")
                VT = itp.tile([P, w], I32, tag="it", name="VT")
                nc.vector.tensor_copy(IT[:], ch0)
                nc.vector.tensor_scalar(IT[:], IT[:], 8388608, 1065353216,
                                        A.mult, A.add)
                nc.vector.tensor_copy(VT[:], IT[:].bitcast(F32))
                tgt = out_tile if out_slice is None else out_slice
                nc.vector.tensor_scalar(tgt, VT[:], MBITS, 0,
                                        A.bitwise_and, A.is_gt)

            def loads(i):
                b, t = iters[i]
                hs = slice(t * P, (t + 1) * P)
                s = st[i]
                s['TD'] = tdp.tile([P, NTD, WH], F32, tag="td", name=f"TD{i}")
                s['TP'] = tpp.tile([P, NPOOL, WH], F32, tag="tp", name=f"TP{i}")
                s['RAND'] = rp.tile([P, w], F32, tag="rand", name=f"RAND{i}")
                TD, TP = s['TD'], s['TP']
                for d0, d1, t0 in TD_RUNS:
                    nc.sync.dma_start(
                        TD[:, t0:t0 + (d1 - d0), MAIN],
                        world[b, d0:d1, hs, :].rearrange("c p w -> p c w"))
                for d0, d1, t0 in TP_RUNS:
                    nc.sync.dma_start(
                        TP[:, t0:t0 + (d1 - d0), MAIN],
                        world[b, d0:d1, hs, :].rearrange("c p w -> p c w"))
                nc.sync.dma_start(s['RAND'][:], rand[b, hs, :])

            def prep(i):
                """EN channel + halo columns of the loaded tiles.  The EN
                compute (DVE) and halo copies (Act) depend only on load(i)."""
                s = st[i]
                TD, TP = s['TD'], s['TP']
                cid = TD[:, 0, MAIN]
                E = mk.tile([P, w], F32, tag="mk", name="E")
                membership(cid, E)
                # EN = (E & didg<0.5) | id>13.5
                nc.vector.scalar_tensor_tensor(E[:], TD[:, 4, MAIN], 0.5,
                                               E[:], A.is_lt, A.logical_and)
                nc.vector.scalar_tensor_tensor(TD[:, ENC, MAIN], cid, 13.5,
                                               E[:], A.is_gt, A.logical_or)
                nc.scalar.copy(TD[:, :, 0:1], TD[:, :, w:w + 1])
                nc.scalar.copy(TD[:, :, w + 1:w + 2], TD[:, :, 1:2])
                nc.scalar.copy(TP[:, :, 0:1], TP[:, :, w:w + 1])
                nc.scalar.copy(TP[:, :, w + 1:w + 2], TP[:, :, 1:2])

            def base1(i):
                # Act: TN <- TD; no mask dependence, overlaps the mask chain
                s = st[i]
                s['TN'] = tnp.tile([P, NTD, WH], F32, tag="tn", name=f"TN{i}")
                nc.scalar.copy(s['TN'][:, 0:6, :], s['TD'][:, 0:6, :])
                nc.scalar.copy(s['TN'][:, 6:NTD, :], s['TD'][:, 6:NTD, :])

            def base2_chunk(i, which):
                # Act: TD <- TN (skipping the EN slot); emitted per chunk
                # right after that chunk's pass-1 CPs and halos so the big
                # copies interleave with CP progress instead of blocking.
                s = st[i]
                if which == 0:
                    nc.scalar.copy(s['TD'][:, 0:5, :], s['TN'][:, 0:5, :])
                else:
                    nc.scalar.copy(s['TD'][:, 6:NTD, :], s['TN'][:, 6:NTD, :])

            def mask_pass(i, p2):
                """a-mask for a pass.  p2=False: fall-left (neighbor j-1,
                overlap shift +1); p2=True: fall-right (neighbor j+1,
                overlap shift -1)."""
                s = st[i]
                cur = s['TD'] if not p2 else s['TN']
                nbr = LEFT if not p2 else RIGHT
                RAND = s['RAND']
                FS = mk.tile([P, w], F32, tag="mk", name="FS")
                DN = mk.tile([P, w], F32, tag="mk", name="DN")
                GB = mk.tile([P, w], F32, tag="mk", name="GB")
                LH = mh.tile([P, WH], F32, tag="mkh", name="LH")
                AM8 = amp.tile([P, WH], I8, tag="am8", name=f"AM8{p2}_{i}")

                # --- leaf merges on DVE (fills its wait-for-chain window),
                # the dependent merges on Pool ----------------------------
                nc.vector.tensor_tensor(DN[:], cur[:, 1, MAIN], cur[:, 1, nbr],
                                        A.is_gt)
                nc.vector.tensor_tensor(GB[:], cur[:, 2, MAIN], cur[:, 2, nbr],
                                        A.logical_and)
                nc.gpsimd.tensor_tensor(FS[:], RAND[:], cur[:, 3, MAIN],
                                        A.add)
                if p2:
                    # + nfm = 2*b1 = 2*a1(j+1)
                    nc.gpsimd.scalar_tensor_tensor(
                        FS[:], s['A18'][:, RIGHT], 2.0, FS[:], A.mult, A.add)
                cmp_op = A.is_gt if not p2 else A.is_le
                nc.gpsimd.scalar_tensor_tensor(DN[:], FS[:], 0.5, DN[:],
                                               cmp_op, A.logical_and)
                nc.gpsimd.tensor_tensor(DN[:], DN[:], cur[:, ENC, MAIN],
                                        A.logical_and)
                nc.gpsimd.tensor_tensor(LH[:, MAIN], DN[:], GB[:],
                                        A.logical_and)
                # --- halo of L, overlap removal -> int8 mask, all on DVE
                # (tiny copies stay off the Act queue, which holds the big
                # base copies and must not head-block the mask finisher)
                which = 2 if p2 else 1
                if not p2:
                    nc.vector.tensor_copy(LH[:, w + 1:w + 2], LH[:, 1:2])
                    nc.vector.scalar_tensor_tensor(AM8[:, MAIN], LH[:, RIGHT],
                                                   0.0, LH[:, MAIN],
                                                   A.is_equal, A.logical_and)
                    nc.vector.tensor_copy(AM8[:, w + 1:w + 2], AM8[:, 1:2])
                    vs = slice(1, w + 2)
                else:
                    nc.vector.tensor_copy(LH[:, 0:1], LH[:, w:w + 1])
                    nc.vector.scalar_tensor_tensor(AM8[:, MAIN], LH[:, LEFT],
                                                   0.0, LH[:, MAIN],
                                                   A.is_equal, A.logical_and)
                    nc.vector.tensor_copy(AM8[:, 0:1], AM8[:, w:w + 1])
                    vs = slice(0, w + 1)
                s[f'A{which}8'] = AM8
                # int32 all-ones mask for the pool XOR channels
                A32 = amp.tile([P, WH], I32, tag="a32", name=f"A32{which}_{i}")
                nc.vector.tensor_scalar(A32[:, vs], AM8[:, vs], -1, None,
                                        A.mult)
                s[f'A32_{which}'] = A32

            def blend_cps(i, p2):
                """Two batched copy_predicated per chunk (DVE), plus halo
                refreshes (Act) split by chunk so pass-2 unblocks early."""
                s = st[i]
                if not p2:
                    src, dst = s['TD'], s['TN']
                    AM8 = s['A18']
                    am, bm = AM8[:, MAIN], AM8[:, RIGHT]   # b[j] = a[j+1]
                    asrc, bsrc = LEFT, RIGHT
                    chunks = ((0, 6, 0), (6, NTD, 6))
                else:
                    src, dst = s['TN'], s['TD']
                    AM8 = s['A28']
                    am, bm = AM8[:, MAIN], AM8[:, LEFT]    # b[j] = a[j-1]
                    asrc, bsrc = RIGHT, LEFT
                    chunks = ((0, 5, 0), (6, NTD, 6))
                for ci, (c0, c1, d0) in enumerate(chunks):
                    nch = c1 - c0
                    d1 = d0 + nch
                    amb = am.unsqueeze(1).broadcast_to((P, nch, w))
                    bmb = bm.unsqueeze(1).broadcast_to((P, nch, w))
                    nc.vector.copy_predicated(dst[:, d0:d1, MAIN], amb,
                                              src[:, c0:c1, asrc])
                    nc.vector.copy_predicated(dst[:, d0:d1, MAIN], bmb,
                                              src[:, c0:c1, bsrc])
                    if not p2:
                        # refresh halos for the pass-2 chain/CPs; mask chunk
                        # first so mask_pass(p2) unblocks early.
                        nc.scalar.copy(dst[:, d0:d1, 0:1],
                                       dst[:, d0:d1, w:w + 1])
                        nc.scalar.copy(dst[:, d0:d1, w + 1:w + 2],
                                       dst[:, d0:d1, 1:2])
                        base2_chunk(i, ci)

            def blend_pool(i, p2):
                """Exact in-place XOR pair-swap of the TP channels.
                pass1 pairs (j-1, j) via a1[j]; pass2 pairs (j, j+1)."""
                s = st[i]
                TP = s['TP']
                which = 2 if p2 else 1
                A32 = s[f'A32_{which}']
                ti = TP[:, :, :].bitcast(I32)
                X = pxp.tile([P, NPOOL, WH], I32, tag="px", name=f"X{which}")
                if not p2:
                    fs, fo = slice(1, w + 2), slice(0, w + 1)
                    osl = RIGHT
                else:
                    fs, fo = slice(0, w + 1), slice(1, w + 2)
                    osl = LEFT
                a32 = A32[:, fs].unsqueeze(1).broadcast_to((P, NPOOL, w + 1))
                nc.gpsimd.tensor_tensor(X[:, :, fs], ti[:, :, fs],
                                        ti[:, :, fo], A.bitwise_xor)
                nc.gpsimd.tensor_tensor(X[:, :, fs], X[:, :, fs], a32,
                                        A.bitwise_and)
                nc.gpsimd.tensor_tensor(ti[:, :, MAIN], ti[:, :, MAIN],
                                        X[:, :, MAIN], A.bitwise_xor)
                nc.gpsimd.tensor_tensor(ti[:, :, MAIN], ti[:, :, MAIN],
                                        X[:, :, osl], A.bitwise_xor)

            def tp_halos(i):
                TP = st[i]['TP']
                nc.scalar.copy(TP[:, :, 0:1], TP[:, :, w:w + 1])
                nc.scalar.copy(TP[:, :, w + 1:w + 2], TP[:, :, 1:2])

            def fixup_store(i):
                b, t = iters[i]
                hs = slice(t * P, (t + 1) * P)
                s = st[i]
                TD, TP = s['TD'], s['TP']
                NF = mk.tile([P, w], F32, tag="mk", name="NF")
                FLI8 = amp.tile([P, w], I8, tag="fli8", name=f"FLI8_{i}")
                # nfm = 2*b1 - 2*b2 = 2*(a1[j+1] - a2[j-1])
                nc.vector.tensor_tensor(NF[:], s['A18'][:, RIGHT],
                                        s['A28'][:, LEFT], A.subtract)
                nc.vector.tensor_scalar(NF[:], NF[:], 2.0, None, A.mult)
                membership(TD[:, 0, MAIN], None, out_slice=FLI8[:])
                nc.vector.copy_predicated(TD[:, 3, MAIN], FLI8[:], NF[:])
                for d0, d1, t0 in TD_RUNS:
                    nc.scalar.dma_start(
                        out[b, d0:d1, hs, :].rearrange("c p w -> p c w"),
                        TD[:, t0:t0 + (d1 - d0), MAIN])
                for d0, d1, t0 in TP_RUNS:
                    nc.scalar.dma_start(
                        out[b, d0:d1, hs, :].rearrange("c p w -> p c w"),
                        TP[:, t0:t0 + (d1 - d0), MAIN])

            # ---- software-pipelined emission -------------------------------
            loads(0)
            prep(0)
            base1(0)
            for i in range(n):
                mask_pass(i, False)
                blend_pool(i, False)
                blend_cps(i, False)     # also emits base2 chunks on Act
                tp_halos(i)
                if i + 1 < n:
                    loads(i + 1)
                    prep(i + 1)
                    base1(i + 1)
                mask_pass(i, True)
                blend_pool(i, True)
                blend_cps(i, True)
                fixup_store(i)

    nc.compile()
    _nc_cache[key] = nc
    return nc


def kernel(world, rand_movement, rand_interact, rand_element):
    del rand_interact, rand_element
    nc = build_kernel()
    in_maps = []
    for k in range(N_CORES):
        bs = slice(k * BPC, (k + 1) * BPC)
        in_maps.append({
            "world": np.ascontiguousarray(world[bs]),
            "rand": np.ascontiguousarray(rand_movement[bs, 0]),
        })
    res = run_bass_kernel_spmd(nc, in_maps, list(range(N_CORES)))
    return np.concatenate([res.results[k]["out"] for k in range(N_CORES)], axis=0)
